# revision 64
# baseline (speedup 1.0000x reference)
"""Multi-head self-attention (b=4, s=2048, d_model=1024, h=16, causal) on 8 trn2 cores.

Sharding: core c = (batch b = c//2, head-group g = c%2): 8 heads of one batch
per core, full QKV + causal attention + partial W_o projection on device; host
pre-transposes x/W slices and sums the two partial y's per batch (the W_o
all-reduce done at unshard time).

All matmul operands are fp16 (full PE stream rate; fp32 runs 2-pass HIGH mode
at half rate) with fp32 PSUM accumulation. x and the weights arrive as
host-prearranged [128, ...] DRAM tensors -> one or four DMAs each, ordered so
the first projection matmul only waits on chunk-0 x + pair-0 W_q/W_k.

The exp stream on the scalar engine paces the attention inner loop, so all
other work is interleaved INTO it: after pair p of chunk j's attention,
the kernel injects pair p's Q/K projection for chunk j+1, V tile p for chunk
j+1, and output-projection tile p of chunk j-1. Per-pair softmax-denominator
flow (stage ctx+denom to SBUF, broadcast the denominator row via a DRAM
bounce, reciprocal_approx_fast, normalize) pipelines two key-tiles behind its
pair's attention; the very last pair instead broadcasts on the PE itself
(K=1 fp32 matmul from partition 64 into PSUM) so the endgame chain has no
DMA latency and the PE stays HAM-warm into the final output projection.
Dummy matmuls on a zeroed tile warm the PE clock gate during the prologue
DMAs.

Attention uses the transposed layout S^T[k,q] = K @ Q^T with the two heads of
a pair row-packed via tile_position (0,0)/(64,0) (auto from base_partition) so
both K=64 score matmuls run concurrently in the PE array. V carries an
appended ones column so denominators fall out of the attn@V matmul (row 64).
Causality: block skip + column restriction + one triangular strip mask.
Diagonal-tile exps run as one strided [128, 2, w] activation. attn@V matmuls
for key-tile i are deferred past tile i+1's scores so the PE never stalls on
the tail exps.
"""

import numpy as np

import concourse.bass as bass
import concourse.tile as tile
from concourse import bacc, mybir
from concourse.bass import ts
from concourse.bass_utils import run_bass_kernel_spmd

F32 = mybir.dt.float32
F16 = mybir.dt.float16

B = 4
S = 2048
DM = 1024
DK = 64
N_CORES = 8
H = 8
PAIRS = 4
NKT = DM // 128   # 8 contraction tiles
NQC = S // 512    # 4 query chunks
AUG = DK + 1      # 65


def _kernel_body(ctx, tc):
    nc = tc.nc
    # host-prearranged inputs (see kernel() for layouts)
    xtr = nc.dram_tensor("xtr", [128, NKT, S], F16, kind="ExternalInput").ap()
    wqr = nc.dram_tensor("wqr", [128, PAIRS * 1024], F16, kind="ExternalInput").ap()
    wkr = nc.dram_tensor("wkr", [128, PAIRS * 1024], F16, kind="ExternalInput").ap()
    wvr = nc.dram_tensor("wvr", [128, NKT * 512], F16, kind="ExternalInput").ap()
    wor = nc.dram_tensor("wor", [128, PAIRS * DM], F16, kind="ExternalInput").ap()
    tri = nc.dram_tensor("tri", [128, 128], F16, kind="ExternalInput").ap()
    y = nc.dram_tensor("y", [S, DM], F16, kind="ExternalOutput").ap()

    outer = ctx.enter_context(tc.tile_pool(name="outer", bufs=1))
    xt_all = outer.tile([128, NKT * S], F16, tag="xall", name="xall")
    xt3 = xt_all.rearrange("p (i s) -> p i s", s=S)
    wq_sb = outer.tile([128, PAIRS * 1024], F16, tag="wq", name="wq")
    wk_sb = outer.tile([128, PAIRS * 1024], F16, tag="wk", name="wk")
    wv_sb = outer.tile([128, NKT * 512], F16, tag="wv", name="wv")
    wo_sb = outer.tile([128, PAIRS * DM], F16, tag="wo", name="wo")
    tri_sb = outer.tile([128, 128], F16, tag="tri", name="tri")
    ones1 = outer.tile([128, 1], F16, tag="ones1", name="ones1")
    # row 64 serves as the K=1 lhsT for the PE denominator broadcast (its
    # base partition must match the stg denominator row's partition 64)
    onesb = outer.tile([128, DK], F32, tag="onesb", name="onesb")
    # DRAM bounce rows for the hidden-path denominator broadcast
    G_dram = outer.tile([32, 512], F32, tag="Gd", name="Gd", space="DRAM")
    kT = [outer.tile([128, S], F16, tag=f"kT{p}", name=f"kT{p}")
          for p in range(PAIRS)]
    v_sb = [outer.tile([128, H * AUG], F16, tag=f"v{t}", name=f"v{t}")
            for t in range(4 * NQC)]

    # prologue DMA order: the first projection matmuls stream per-i-block so
    # compute chases the DMAs; x block 0 + pair-0 W_q/W_k first.
    nc.sync.dma_start(out=xt3[:, 0, ts(0, 512)], in_=xtr[:, 0, ts(0, 512)])
    nc.sync.dma_start(out=wq_sb[:, ts(0, 1024)], in_=wqr[:, ts(0, 1024)])
    nc.sync.dma_start(out=wk_sb[:, ts(0, 1024)], in_=wkr[:, ts(0, 1024)])
    for i in range(1, NKT):
        nc.sync.dma_start(out=xt3[:, i, ts(0, 512)], in_=xtr[:, i, ts(0, 512)])
    for p in range(1, PAIRS):
        nc.sync.dma_start(out=wq_sb[:, ts(p, 1024)], in_=wqr[:, ts(p, 1024)])
        nc.sync.dma_start(out=wk_sb[:, ts(p, 1024)], in_=wkr[:, ts(p, 1024)])
    nc.sync.dma_start(out=wv_sb, in_=wvr)
    for j in range(1, NQC):
        nc.sync.dma_start(out=xt3[:, :, ts(j, 512)], in_=xtr[:, :, ts(j, 512)])
    nc.sync.dma_start(out=tri_sb, in_=tri)
    nc.sync.dma_start(out=wo_sb, in_=wor)
    warm = outer.tile([128, 512], F16, tag="warm", name="warm")
    nc.vector.memset(warm[:], 0.0)
    nc.vector.memset(ones1[:], 1.0)
    nc.vector.memset(onesb[:], 1.0)

    qcp = ctx.enter_context(tc.tile_pool(name="qcp", bufs=3))
    ap_ = ctx.enter_context(tc.tile_pool(name="attn", bufs=4))
    sgp = ctx.enter_context(tc.tile_pool(name="sgp", bufs=8))
    r2p = ctx.enter_context(tc.tile_pool(name="r2p", bufs=6))
    cxp = ctx.enter_context(tc.tile_pool(name="cxp", bufs=3))
    yp = ctx.enter_context(tc.tile_pool(name="yp", bufs=3))
    ps_w = ctx.enter_context(tc.tile_pool(name="psw", bufs=2, space="PSUM"))
    ps_s = ctx.enter_context(tc.tile_pool(name="pscore", bufs=2, space="PSUM"))
    ps_o = ctx.enter_context(tc.tile_pool(name="pout", bufs=1, space="PSUM"))

    def _proj_pair(j, p, qc_list):
        xoff = j * 512
        psq = ps_w.tile([128, 512], F32, tag="ps", name="ps")
        for i in range(NKT):
            nc.tensor.matmul(psq[:],
                             wq_sb[:, p * 1024 + i * 128:p * 1024 + i * 128 + 128],
                             xt_all[:, i * S + xoff:i * S + xoff + 512],
                             start=(i == 0), stop=(i == NKT - 1))
        q_ = qcp.tile([128, 512], F16, tag=f"qc{p}", name=f"qc{p}")
        nc.vector.tensor_copy(q_[:], psq[:])
        qc_list.append(q_)
        psk = ps_w.tile([128, 512], F32, tag="ps", name="ps")
        for i in range(NKT):
            nc.tensor.matmul(psk[:],
                             wk_sb[:, p * 1024 + i * 128:p * 1024 + i * 128 + 128],
                             xt_all[:, i * S + xoff:i * S + xoff + 512],
                             start=(i == 0), stop=(i == NKT - 1))
        nc.scalar.copy(kT[p][:, ts(j, 512)], psk[:])

    def _vproj(j, tt):
        xoff = j * 512
        t = 4 * j + tt
        psv = ps_w.tile([128, 512], F32, tag="ps", name="ps")
        for i in range(NKT):
            nc.tensor.matmul(psv[:],
                             xt_all[:, i * S + xoff + tt * 128:
                                   i * S + xoff + tt * 128 + 128],
                             wv_sb[:, ts(i, 512)],
                             start=(i == 0), stop=(i == NKT - 1))
        vt = v_sb[t]
        nc.vector.tensor_copy(
            vt[:].rearrange("p (h a) -> p h a", a=AUG)[:, :, 0:DK],
            psv[:].rearrange("p (h a) -> p h a", a=DK))
        ones_col = vt[:].rearrange("p (h a) -> p h a", a=AUG)[:, :, DK]
        nc.vector.tensor_copy(ones_col, ones1[:].to_broadcast((128, H)))

    def _emit_half(cxc, jj, tt, oc, ysb, tail=False):
        t = 4 * jj + tt
        psy = ps_w.tile([128, 512], F32, tag="ps", name="ps")
        for p in range(PAIRS):
            nc.tensor.matmul(psy[:], cxc[p][:, ts(tt, 128)],
                             wo_sb[:, p * DM + oc * 512:
                                   p * DM + oc * 512 + 512],
                             start=(p == 0), stop=(p == PAIRS - 1))
        if tail and oc == 0:
            # scalar-engine copy so the two halves' copies run in parallel
            nc.scalar.copy(ysb[:, ts(oc, 512)], psy[:])
        else:
            nc.vector.tensor_copy(ysb[:, ts(oc, 512)], psy[:])
        nc.sync.dma_start(out=y[ts(t, 128), ts(oc, 512)],
                          in_=ysb[:, ts(oc, 512)])

    def _emit_tile(cxc, jj, tt, tail=False):
        ysb = yp.tile([128, DM], F16, tag="y", name="ysb")
        for oc in range(2):
            _emit_half(cxc, jj, tt, oc, ysb, tail=tail)

    # dummy matmuls on the zeroed warm tile keep the PE HAM activity monitor
    # busy while the first DMAs land, so chunk-0 runs at full clock
    for _ in range(18):
        pswm = ps_w.tile([128, 512], F32, tag="ps", name="ps")
        nc.tensor.matmul(pswm[:], warm[:, 0:128], warm[:],
                         start=True, stop=True)

    # chunk-0 projections run up front (nothing to overlap them with yet)
    qc_cur = []
    for p in range(PAIRS):
        _proj_pair(0, p, qc_cur)
    for tt in range(4):
        _vproj(0, tt)

    pending = None   # (jj, cxc) for the previous chunk's output projection
    carry = None         # (pa, pi, pc0, oa, ob, ha, hb, p, jd, cxl)
    norm_pending = None  # (pair_stg, p, jd, cxl) awaiting normalize

    def _drain_norm(pair_stg, p_, jd, cxl):
        cx = cxp.tile([128, 512], F16, tag=f"cx{p_}", name=f"cx{p_}")
        tail = (jd == NQC - 1 and p_ == PAIRS - 1)
        for hs in range(2):
            if tail:
                # PE broadcast of the denominator row (K=1 fp32 matmul
                # from partition 64): no DMA latency on the endgame chain
                r2 = ps_w.tile([128, 512], F32, tag="ps", name="ps")[0:DK, :]
                nc.tensor.matmul(r2, onesb[64:65, :],
                                 pair_stg[hs][DK:AUG, :],
                                 start=True, stop=True)
            else:
                # DRAM-bounce broadcast: zero PE cost, hidden under the
                # next pair's attention
                g_row = jd * 8 + p_ * 2 + hs
                r2 = r2p.tile([64, 512], F32, tag="R2", name="R2")[:]
                nc.sync.dma_start(
                    out=r2,
                    in_=G_dram[g_row:g_row + 1, :].to_broadcast((64, 512)))
            nc.vector.reciprocal_approx_fast(r2, r2)
            nc.vector.tensor_mul(cx[64 * hs:64 * hs + 64, :],
                                 pair_stg[hs][0:DK, :], r2)
        cxl.append(cx)

    def _drain(c):
        pa_, pi_, pc0_, oa_, ob_, ha_, hb_, p_, jd, cxl = c
        nc.tensor.matmul(oa_[0:AUG, pc0_:512],
                         v_sb[pi_][:, ha_ * AUG:(ha_ + 1) * AUG],
                         pa_[:, pc0_:512], start=(pi_ == 0), stop=True)
        nc.tensor.matmul(ob_[0:AUG, pc0_:512],
                         v_sb[pi_][:, hb_ * AUG:(hb_ + 1) * AUG],
                         pa_[:, 512 + pc0_:1024], start=(pi_ == 0), stop=True)
        pair_stg = []
        for (o_ps, hs) in ((oa_, 0), (ob_, 1)):
            stg = sgp.tile([AUG, 512], F32, tag="stg", name="stg")
            nc.vector.tensor_copy(stg[:], o_ps[:])
            if not (jd == NQC - 1 and p_ == PAIRS - 1):
                g_row = jd * 8 + p_ * 2 + hs
                nc.sync.dma_start(out=G_dram[g_row:g_row + 1, :],
                                  in_=stg[DK:AUG, :])
            pair_stg.append(stg)
        return (pair_stg, p_, jd, cxl)

    for j in range(NQC):
        cx_list = []
        qc_next = []
        if 1 <= j and j + 1 < NQC:
            # boundary filler: V tiles 0-1 of chunk j+1 (inputs resident
            # since the prologue) keep the PE busy while the previous
            # chunk's final exps free the score-PSUM buffers
            _vproj(j + 1, 0)
            _vproj(j + 1, 1)

        for p in range(PAIRS):
            ha, hb = 2 * p, 2 * p + 1
            nk = 4 * j + 4
            oa = ps_o.tile([AUG, 512], F32, tag="oa", name="oa")
            ob = ps_o.tile([AUG, 512], F32, tag="ob", name="ob")
            # last chunk has no next-chunk projection to inject, so spread the
            # previous chunk's output-projection tile one matmul per key-tile
            fine_emit = (j + 1 == NQC and pending is not None)
            if fine_emit:
                jj0, cxc0 = pending
                ysb_cur = yp.tile([128, DM], F16, tag="y", name="ysb")
                psys = [ps_w.tile([128, 512], F32, tag="ps", name="ps")
                        for _ in range(2)]
                emit_ops = [('mm', oc, pp) for oc in range(2)
                            for pp in range(PAIRS)]
                emit_ops.insert(4, ('fin', 0, None))
                emit_ops.append(('fin', 1, None))
                if True:
                    # pair-start filler: the first three accumulation matmuls
                    # depend only on long-ready cx pairs 0-2 of chunk j-1;
                    # they bridge the pair-transition window where only the
                    # previous pair's drain matmuls are otherwise available
                    for _ in range(3):
                        kind, oc, pp = emit_ops.pop(0)
                        nc.tensor.matmul(
                            psys[oc][:], cxc0[pp][:, ts(p, 128)],
                            wo_sb[:, pp * DM + oc * 512:
                                  pp * DM + oc * 512 + 512],
                            start=(pp == 0), stop=(pp == PAIRS - 1))
            prev = None
            for i in range(nk):
                d = i - 4 * j
                c0 = 128 * d if d > 0 else 0
                w = 512 - c0
                at = ap_.tile([128, 1024], F16, tag="at", name="at")
                sp = ps_s.tile([128, 1024], F32, tag="sp", name="sp")
                nc.tensor.matmul(sp[0:128, c0:512],
                                 kT[p][0:64, ts(i, 128)],
                                 qc_cur[p][0:64, bass.ds(c0, w)],
                                 start=True, stop=True)
                nc.tensor.matmul(sp[0:128, 512 + c0:1024],
                                 kT[p][64:128, ts(i, 128)],
                                 qc_cur[p][64:128, bass.ds(c0, w)],
                                 start=True, stop=True)
                if carry is not None:
                    norm_pending = _drain(carry)
                    carry = None
                elif norm_pending is not None and i >= 2:
                    _drain_norm(*norm_pending)
                    norm_pending = None
                if c0 == 0:
                    nc.scalar.activation(at[:], sp[:],
                                         mybir.ActivationFunctionType.Exp,
                                         scale=0.125)
                else:
                    sp_strip = bass.AP(tensor=sp.tensor, offset=sp.offset + c0,
                                       ap=[list(sp.ap[0]), [512, 2], [1, w]])
                    at_strip = bass.AP(tensor=at.tensor, offset=at.offset + c0,
                                       ap=[list(at.ap[0]), [512, 2], [1, w]])
                    nc.scalar.activation(at_strip, sp_strip,
                                         mybir.ActivationFunctionType.Exp,
                                         scale=0.125)
                if d >= 0:
                    strip = bass.AP(tensor=at.tensor, offset=at.offset + c0,
                                    ap=[list(at.ap[0]), [512, 2], [1, 128]])
                    tri_b = bass.AP(tensor=tri_sb.tensor, offset=tri_sb.offset,
                                    ap=[list(tri_sb.ap[0]), [0, 2], [1, 128]])
                    nc.vector.tensor_mul(strip, strip, tri_b)
                if prev is not None:
                    pa, pi, pc0 = prev
                    nc.tensor.matmul(oa[0:AUG, pc0:512],
                                     v_sb[pi][:, ha * AUG:(ha + 1) * AUG],
                                     pa[:, pc0:512], start=(pi == 0), stop=False)
                    nc.tensor.matmul(ob[0:AUG, pc0:512],
                                     v_sb[pi][:, hb * AUG:(hb + 1) * AUG],
                                     pa[:, 512 + pc0:1024], start=(pi == 0),
                                     stop=False)
                if fine_emit and i >= 4 and emit_ops:
                    kind, oc, pp = emit_ops.pop(0)
                    if kind == 'mm':
                        nc.tensor.matmul(
                            psys[oc][:], cxc0[pp][:, ts(p, 128)],
                            wo_sb[:, pp * DM + oc * 512:
                                  pp * DM + oc * 512 + 512],
                            start=(pp == 0), stop=(pp == PAIRS - 1))
                    else:
                        nc.vector.tensor_copy(ysb_cur[:, ts(oc, 512)],
                                              psys[oc][:])
                        nc.sync.dma_start(
                            out=y[ts(4 * jj0 + p, 128), ts(oc, 512)],
                            in_=ysb_cur[:, ts(oc, 512)])
                prev = (at, i, c0)
            pa, pi, pc0 = prev
            carry = (pa, pi, pc0, oa, ob, ha, hb, p, j, cx_list)
            # interleave next-chunk projections and previous-chunk output
            # projection into the ACT-paced attention stretch
            if j + 1 < NQC:
                _proj_pair(j + 1, p, qc_next)
                if j == 0:
                    _vproj(j + 1, p)
                elif p < PAIRS - 2:
                    _vproj(j + 1, p + 2)
            if pending is not None and not fine_emit:
                _emit_tile(pending[1], pending[0], p)
        _drain_norm(*_drain(carry))
        carry = None

        pending = (j, cx_list)
        qc_cur = qc_next
        if j == NQC - 1:
            for tt in range(4):
                _emit_tile(cx_list, j, tt, tail=True)

_NC_CACHE = None


def _build():
    global _NC_CACHE
    if _NC_CACHE is None:
        from contextlib import ExitStack
        nc = bacc.Bacc("TRN2", target_bir_lowering=False, debug=False,
                       num_devices=N_CORES)
        with tile.TileContext(nc) as tc:
            with ExitStack() as ctx:
                _kernel_body(ctx, tc)
        nc.compile()
        _NC_CACHE = nc
    return _NC_CACHE


def _make_tri():
    K = np.arange(128)[:, None]
    Q = np.arange(128)[None, :]
    return (Q >= K).astype(np.float16)


def kernel(x, W_q, W_k, W_v, W_o, _trace=False, _tmpdir=None):
    x = np.asarray(x, dtype=np.float32)
    tri = _make_tri()
    f16 = np.float16

    def _wblk_i(W, rows):
        # i-major: [128, 8*512] with contraction block i at cols i*512..
        wT = np.ascontiguousarray(np.asarray(W)[rows, :].T)  # [1024, 512]
        return np.ascontiguousarray(
            wT.reshape(NKT, 128, 512).transpose(1, 0, 2).reshape(128, NKT * 512)
        ).astype(f16)

    def _wblk_p(W, rows):
        # pair-major: [128, p*1024 + i*128 + c]
        wT = np.ascontiguousarray(np.asarray(W)[rows, :].T)  # [1024, 512]
        return np.ascontiguousarray(
            wT.reshape(NKT, 128, PAIRS, 128).transpose(1, 2, 0, 3).reshape(
                128, PAIRS * 1024)).astype(f16)

    in_maps = []
    for c in range(N_CORES):
        b, g = divmod(c, 2)
        rows = slice(512 * g, 512 * (g + 1))
        xT = np.ascontiguousarray(x[b].T)  # [1024, 2048]
        xtr = np.ascontiguousarray(
            xT.reshape(NKT, 128, S).transpose(1, 0, 2)).astype(f16)
        woT = np.ascontiguousarray(np.asarray(W_o)[:, rows].T)  # [512, 1024]
        wor = np.ascontiguousarray(
            woT.reshape(PAIRS, 128, DM).transpose(1, 0, 2).reshape(
                128, PAIRS * DM)).astype(f16)
        in_maps.append({
            "xtr": xtr,
            "wqr": _wblk_p(W_q, rows),
            "wkr": _wblk_p(W_k, rows),
            "wvr": _wblk_i(W_v, rows),
            "wor": wor,
            "tri": tri,
        })
    nc = _build()
    res = run_bass_kernel_spmd(nc, in_maps, core_ids=list(range(N_CORES)),
                               trace=_trace, tmpdir=_tmpdir)
    out = np.stack([res.results[2 * b]["y"].astype(np.float32)
                    + res.results[2 * b + 1]["y"].astype(np.float32)
                    for b in range(B)])
    kernel._last_exec_time_ns = res.exec_time_ns
    kernel._last_results = res
    return out
